# revision 14
# baseline (speedup 1.0000x reference)
"""Longformer layer (B=2, S=4096, D=768, H=12, w=128, NG=32) on 8 TRN2 cores.

Sharding: sequence-parallel. Core c owns tokens [q0, q0+1024) of batch b=c//4,
q0 = (c%4)*1024. Each core computes band+global-key attention and the dense
pipeline for its 1024 tokens. Global-QUERY rows (tokens 0..NG of each batch)
need keys from every core, so each core also emits flash-style partials
(sum exp*v and sum exp over its local keys); the host combines those and runs
the dense tail for the 2*NG global rows during gather/unshard.

v2 speedups over the working baseline:
- QKV / Wo / Wo2 projections run as fp8(e4m3) DoubleRow matmuls (weights
  pre-scaled x64 on the host; dequant via the ACT scale on PSUM read).
- Band-score matmuls for head pairs issue at tile_position rows 0/64 and
  overlap in the PE array; global-key scores stack 4 heads per PSUM tile
  (rows x cols tiling) and their exp batches 4 heads per ACT op.
- The LN scale/shift no longer burns TensorE broadcasts: mu/rstd rows are
  partition-broadcast on GPSIMD and applied with DVE scalar_tensor_tensor.
- The attention softmax divide batches per head: denominator row from the
  v ones-column (=1/16 so ctx lands pre-scaled x16 for fp8), reciprocal,
  GPSIMD broadcast, one DVE multiply into the fp8 ctx tile.
"""

import math
import numpy as np
import ml_dtypes

import concourse.bacc as bacc
import concourse.mybir as mybir
import concourse.tile as tile
from concourse.masks import make_identity

F32 = mybir.dt.float32
F32R = mybir.dt.float32r
BF16 = mybir.dt.bfloat16
FP8 = mybir.dt.float8e4
AF = mybir.ActivationFunctionType
ALU = mybir.AluOpType
DR = mybir.MatmulPerfMode.DoubleRow

B, S, D, FF = 2, 4096, 768, 3072
H, DH, W, NG = 12, 64, 128, 32
EPS = 1e-12
T = 1024           # owned tokens per core
TH = T + 2 * W     # with halo
KD = D // 128      # 6 feature tiles
KF = FF // 128     # 24
NCH = T // W       # 8 owned chunks
NJ = NCH + 2       # k-chunks j=-1..8  (jdx = j+1)
HALF = 512
N_CORES = 8
ISCALE = 1.0 / math.sqrt(DH)
W8S = 64.0         # host pre-scale on fp8 weights
CTXS = 16.0        # ctx fp8 scale (via v ones-col = 1/16)

# which FFN matmuls run fp8 DoubleRow (accuracy-gated; see module docstring)
WI_FP8 = False
WO2_FP8 = True

_nc_cache = {}


def r(ap):
    return ap.bitcast(F32R)


def build_body(nc, tc, ap, ctx, sim_mode=False, stop_after=None):
    def dummy_out(pool, og_too=True):
        z = pool.tile([128, T], F32, tag="zdum", name="zdum")
        nc.vector.memset(z, 0.0)
        for k in range(KD):
            nc.sync.dma_start(out=ap["outT"][k * 128:(k + 1) * 128, :], in_=z)
        if og_too:
            zg = pool.tile([65, NG], F32, tag="zgdum", name="zgdum")
            nc.vector.memset(zg, 1.0)
            for h in range(H):
                nc.sync.dma_start(out=ap["og"][h], in_=zg)
    gelu_f = AF.Identity if sim_mode else AF.Gelu
    persist = ctx.enter_context(tc.tile_pool(name="persist", bufs=1))

    # ---------------- constants / biases -----------------------------------
    ident = persist.tile([128, 128], F32, tag="ident", name="ident")
    make_identity(nc, ident)
    ones_col = persist.tile([128, 1], F32, tag="ones_col", name="ones_col")
    nc.vector.memset(ones_col, 1.0)
    nc.vector.tensor_scalar_mul(out=ones_col[:].bitcast(F32R), in0=ones_col,
                                scalar1=1.0)
    eps_sc = persist.tile([1, 1], F32, tag="eps_sc", name="eps_sc")
    nc.vector.memset(eps_sc, EPS)

    def load_bias_cols(name, n):
        t = persist.tile([128, n], F32, tag=name)
        nc.sync.dma_start(out=t, in_=ap[name].rearrange("(k p) -> p k", p=128))
        return t

    bq_sb = load_bias_cols("bq", KD)
    bk_sb = load_bias_cols("bk", KD)
    bo_sb = load_bias_cols("bo", KD)
    bi_sb = load_bias_cols("bi", KF)
    bo2_sb = load_bias_cols("bo2", KD)   # host pre-adds ln1_b
    g1_sb = load_bias_cols("ln1_g", KD)
    b1_sb = load_bias_cols("ln1_b", KD)
    g2_sb = load_bias_cols("ln2_g", KD)
    b2_sb = load_bias_cols("ln2_b", KD)

    bv_bc = persist.tile([128, D], F32, tag="bv_bc", name="bv_bc")
    nc.gpsimd.dma_start(out=bv_bc, in_=ap["bv"].unsqueeze(0).partition_broadcast(128))

    am_sb = persist.tile([128, NJ], F32, tag="am_sb", name="am_sb")
    nc.sync.dma_start(out=am_sb, in_=ap["am_halo"].rearrange("(k p) -> p k", p=128))
    amg4_sb = persist.tile([128, 1], F32, tag="amg4_sb", name="amg4_sb")
    nc.sync.dma_start(out=amg4_sb, in_=ap["am_glob4"].unsqueeze(1))

    # attn_out outlives the attention scope
    pool_ao = ctx.enter_context(tc.tile_pool(name="pool_ao", bufs=1))
    attn_out = [pool_ao.tile([128, T], F32, tag=f"ao{k}", name=f"ao{k}")
                for k in range(KD)]
    attn_outB = pool_ao.tile([128, KD, T], FP8 if WI_FP8 else BF16,
                             tag="aob", name="aob")

    # ---------------- layernorm helper --------------------------------------
    def layernorm(u_tiles, g_col, b_col, dest_aps, ln_sb, row_ps):
        """u_tiles: KD SBUF tiles [128, HALF] f32 feature-major.
        dest_aps[k] <- LN(u)*g (+b if b_col is not None)."""
        s1 = row_ps.tile([1, HALF], F32, tag="s1", name="s1")
        s2 = row_ps.tile([1, HALF], F32, tag="s2", name="s2")
        for k in range(KD):
            nc.tensor.matmul(s1[:], r(ones_col), r(u_tiles[k][:]),
                             start=(k == 0), stop=(k == KD - 1))
        for k in range(KD):
            usq = ln_sb.tile([128, HALF], F32, tag="usq", name="usq", bufs=2)
            nc.vector.tensor_mul(out=usq[:].bitcast(F32R), in0=u_tiles[k][:],
                                 in1=u_tiles[k][:])
            nc.tensor.matmul(s2[:], r(ones_col), r(usq[:]),
                             start=(k == 0), stop=(k == KD - 1))
        mu = ln_sb.tile([1, HALF], F32, tag="mu", name="mu")
        nc.vector.tensor_scalar_mul(out=mu, in0=s1, scalar1=1.0 / D)
        var = ln_sb.tile([1, HALF], F32, tag="var", name="var")
        nc.vector.tensor_scalar_mul(out=var, in0=s2, scalar1=1.0 / D)
        musq = ln_sb.tile([1, HALF], F32, tag="musq", name="musq")
        nc.vector.tensor_mul(out=musq, in0=mu, in1=mu)
        nc.vector.tensor_sub(out=var, in0=var, in1=musq)
        sd = ln_sb.tile([1, HALF], F32, tag="sd", name="sd")
        nc.scalar.activation(out=sd, in_=var, func=AF.Sqrt, bias=eps_sc[:])
        rstd = ln_sb.tile([1, HALF], F32, tag="rstd", name="rstd")
        with nc.allow_low_precision(reason="f32r rounding only"):
            nc.vector.reciprocal(out=rstd[:].bitcast(F32R), in_=sd)
        mu_bc = ln_sb.tile([128, HALF], F32, tag="mu_bc", name="mu_bc", bufs=2)
        rstd_bc = ln_sb.tile([128, HALF], F32, tag="rstd_bc", name="rstd_bc",
                             bufs=2)
        nc.gpsimd.partition_broadcast(mu_bc[:], mu[:])
        nc.gpsimd.partition_broadcast(rstd_bc[:], rstd[:])
        for k in range(KD):
            dtile = ln_sb.tile([128, HALF], F32, tag="d", name="d", bufs=2)
            nc.vector.tensor_sub(out=dtile[:].bitcast(F32R), in0=u_tiles[k][:],
                                 in1=mu_bc)
            nc.vector.scalar_tensor_tensor(
                out=dest_aps[k].bitcast(F32R), in0=dtile[:],
                scalar=g_col[:, k:k + 1], in1=rstd_bc[:],
                op0=ALU.mult, op1=ALU.mult)
            if b_col is not None:
                nc.vector.tensor_scalar_add(out=dest_aps[k], in0=dest_aps[k],
                                            scalar1=b_col[:, k:k + 1])

    with tc.tile_pool(name="pool_x", bufs=1) as pool_x, \
         tc.tile_pool(name="pool_ctx", bufs=1) as pool_ctx:
        xT = [pool_x.tile([128, TH], F32, tag=f"xT{k}", name=f"xT{k}")
              for k in range(KD)]
        x8 = pool_x.tile([128, KD, TH], FP8, tag="x8", name="x8")
        xgT4 = pool_x.tile([128, KD, 128], FP8, tag="xgT4", name="xgT4")
        ctx8 = pool_ctx.tile([128, KD, T], FP8, tag="ctx8", name="ctx8")

        with tc.tile_pool(name="pool_qkv", bufs=1) as pool_qkv:
            # ---------------- load x, transpose to feature-major ------------
            with tc.tile_pool(name="xload", bufs=3) as xload, \
                 tc.tile_pool(name="tp_ps", bufs=3, space="PSUM") as tp_ps:
                for ti in range(TH // 128):
                    xtile = xload.tile([128, D], F32, tag="xtile", name="xtile")
                    nc.sync.dma_start(out=xtile,
                                      in_=ap["x_halo"][ti * 128:(ti + 1) * 128, :])
                    for k in range(KD):
                        ps = tp_ps.tile([128, 128], F32, tag="tp", name="tp")
                        nc.tensor.transpose(ps[:], xtile[:, k * 128:(k + 1) * 128],
                                            ident[:])
                        nc.scalar.activation(out=xT[k][:, ti * 128:(ti + 1) * 128],
                                             in_=ps, func=AF.Copy)
                        nc.vector.tensor_copy(
                            out=x8[:, k, ti * 128:(ti + 1) * 128], in_=ps)
                xg = xload.tile([NG, D], F32, tag="xg", name="xg")
                nc.sync.dma_start(out=xg, in_=ap["x_glob"])
                for k in range(KD):
                    ps = tp_ps.tile([128, 128], F32, tag="tpg", name="tpg")
                    for rr in range(4):
                        nc.tensor.transpose(ps[:, rr * 32:(rr + 1) * 32],
                                            xg[:, k * 128:(k + 1) * 128],
                                            ident[0:NG, 0:NG])
                    nc.vector.tensor_copy(out=xgT4[:, k, :], in_=ps)

            if stop_after == "x":
                dummy_out(pool_qkv)
                return
            # ---------------- projections (fp8 DoubleRow) -------------------
            qT = [pool_qkv.tile([128, T], BF16, tag=f"qT{k}", name=f"qT{k}")
                  for k in range(KD)]
            kT = [pool_qkv.tile([128, TH], BF16, tag=f"kT{k}", name=f"kT{k}")
                  for k in range(KD)]
            # v: token-major per halo chunk, heads interleaved with 1/16 col:
            # col h*65+d = v[tok, h, d], col h*65+64 = 1/16
            v_sb = [pool_qkv.tile([128, H * 65], BF16, tag=f"v{j}", name=f"v{j}")
                    for j in range(NJ)]
            vg4_sb = pool_qkv.tile([128, H * 65], BF16, tag="vg4", name="vg4")
            qgT = [pool_qkv.tile([128, NG], BF16, tag=f"qgT{k}", name=f"qgT{k}")
                   for k in range(KD)]
            kgT = [pool_qkv.tile([128, NG], BF16, tag=f"kgT{k}", name=f"kgT{k}")
                   for k in range(KD)]
            masks = []
            for j in range(NJ):
                m = pool_qkv.tile([128, 3 * W], BF16, tag=f"mask{j}",
                                  name=f"mask{j}")
                nc.sync.dma_start(out=m, in_=ap["mask_all"][j])
                masks.append(m)

            with tc.tile_pool(name="wload", bufs=2) as wload, \
                 tc.tile_pool(name="proj_ps", bufs=2, space="PSUM") as proj_ps, \
                 tc.tile_pool(name="vproj_ps", bufs=2, space="PSUM") as vproj_ps:
                for wname, bias_sb, dest, gdest, ncols, coff in (
                        ("Wq8", bq_sb, qT, qgT, T, W),
                        ("Wk8", bk_sb, kT, kgT, TH, 0)):
                    wt = wload.tile([128, KD, D], FP8, tag="w", name=f"w{wname}")
                    nc.sync.dma_start(
                        out=wt, in_=ap[wname].rearrange("(k p) o -> p k o", p=128))
                    for o in range(KD):
                        for c0 in range(0, ncols, HALF):
                            cw = min(HALF, ncols - c0)
                            ps = proj_ps.tile([128, HALF], F32, tag="proj",
                                              name="proj")
                            for jp in range(KD // 2):
                                nc.tensor.matmul(
                                    ps[:, :cw],
                                    wt[:, 2 * jp:2 * jp + 2, o * 128:(o + 1) * 128],
                                    x8[:, 2 * jp:2 * jp + 2, coff + c0:coff + c0 + cw],
                                    start=(jp == 0), stop=(jp == KD // 2 - 1),
                                    perf_mode=DR)
                            nc.scalar.activation(out=dest[o][:, c0:c0 + cw],
                                                 in_=ps[:, :cw], func=AF.Identity,
                                                 bias=bias_sb[:, o:o + 1],
                                                 scale=1.0 / W8S)
                        psg = proj_ps.tile([128, NG], F32, tag="projg", name="projg")
                        for k in range(KD):
                            nc.tensor.matmul(psg[:],
                                             wt[:, k, o * 128:(o + 1) * 128],
                                             xgT4[:, k, 0:NG], start=(k == 0),
                                             stop=(k == KD - 1))
                        nc.scalar.activation(out=gdest[o], in_=psg, func=AF.Identity,
                                             bias=bias_sb[:, o:o + 1],
                                             scale=1.0 / W8S)
                wv = wload.tile([128, KD, D], FP8, tag="w", name="wWv8")
                nc.sync.dma_start(
                    out=wv, in_=ap["Wv8"].rearrange("(k p) o -> p k o", p=128))

                def v_project(lhs_pairs, lhs_single, n_tok, dest):
                    ps = vproj_ps.tile([128, D], F32, tag="vproj", name="vproj")
                    for c0 in range(0, D, HALF):
                        cw = min(HALF, D - c0)
                        if lhs_pairs is not None:
                            for jp in range(KD // 2):
                                nc.tensor.matmul(
                                    ps[:n_tok, c0:c0 + cw],
                                    lhs_pairs(jp),
                                    wv[:, 2 * jp:2 * jp + 2, c0:c0 + cw],
                                    start=(jp == 0), stop=(jp == KD // 2 - 1),
                                    perf_mode=DR)
                        else:
                            for k in range(KD):
                                nc.tensor.matmul(
                                    ps[:n_tok, c0:c0 + cw], lhs_single(k),
                                    wv[:, k, c0:c0 + cw],
                                    start=(k == 0), stop=(k == KD - 1))
                    dv = dest[:n_tok].rearrange("p (h e) -> p h e", e=65)
                    nc.vector.scalar_tensor_tensor(
                        out=dv[:, :, 0:64], in0=ps[:n_tok].rearrange(
                            "p (h d) -> p h d", d=DH),
                        scalar=1.0 / W8S, in1=bv_bc[:n_tok].rearrange(
                            "p (h d) -> p h d", d=DH),
                        op0=ALU.mult, op1=ALU.add)
                    nc.vector.memset(dv[:, :, 64:65], 1.0 / CTXS)

                for j in range(NJ):
                    v_project(
                        lambda jp, j=j: x8[:, 2 * jp:2 * jp + 2,
                                           j * 128:(j + 1) * 128],
                        None, 128, v_sb[j])
                v_project(lambda jp: xgT4[:, 2 * jp:2 * jp + 2, :],
                          None, 128, vg4_sb)

            if stop_after == "qkv":
                dummy_out(pool_qkv)
                return
            # ---------------- attention ----------------
            def h_slice(t_list, h, cols):
                return t_list[h // 2][(h % 2) * DH:(h % 2) * DH + DH, cols]

            with tc.tile_pool(name="egsb", bufs=1) as egsb:
                # global-key scores: stack 4 heads per psum tile, batch exp
                eg_sb = [egsb.tile([128, T], BF16, tag=f"eg{g}", name=f"eg{g}")
                         for g in range(3)]
                with tc.tile_pool(name="eg_ps", bufs=2, space="PSUM") as eg_ps:
                    for g in range(3):
                        for c0 in range(0, T, HALF):
                            gps = eg_ps.tile([128, HALF], F32, tag="egp",
                                             name="egp")
                            for h4 in range(4):
                                h = 4 * g + h4
                                nc.tensor.matmul(
                                    gps[h4 * 32:h4 * 32 + 32, :],
                                    h_slice(kgT, h, slice(0, NG)),
                                    h_slice(qT, h, slice(c0, c0 + HALF)),
                                    start=True, stop=True,
                                    tile_position=((h % 2) * 64, h4 * 32))
                            nc.scalar.activation(out=eg_sb[g][:, c0:c0 + HALF],
                                                 in_=gps, func=AF.Exp,
                                                 bias=amg4_sb[:], scale=ISCALE)

                with tc.tile_pool(name="esb", bufs=3) as esb, \
                     tc.tile_pool(name="epsb", bufs=2) as epsb, \
                     tc.tile_pool(name="ogsb", bufs=2) as ogsb, \
                     tc.tile_pool(name="rcsb", bufs=2) as rcsb, \
                     tc.tile_pool(name="bcsb", bufs=2) as bcsb, \
                     tc.tile_pool(name="sc_ps", bufs=1, space="PSUM") as sc_ps, \
                     tc.tile_pool(name="av_ps", bufs=1, space="PSUM") as av_ps, \
                     tc.tile_pool(name="pg_ps", bufs=1, space="PSUM") as pg_ps:
                    for hp in range(H // 2):
                        heads = (2 * hp, 2 * hp + 1)
                        # band scores, transposed; head pair at PE rows 0/64
                        e_tiles = {}
                        for j in range(-1, NCH + 1):
                            jdx = j + 1
                            cs = [c for c in (j - 1, j, j + 1) if 0 <= c < NCH]
                            wj = 128 * len(cs)
                            q_lo = cs[0] * 128
                            pss = []
                            for par, h in enumerate(heads):
                                ps = sc_ps.tile([128, 3 * W], F32,
                                                tag=f"sc{par}", name=f"sc{par}")
                                nc.tensor.matmul(
                                    ps[:, :wj],
                                    h_slice(kT, h, slice(jdx * 128, jdx * 128 + 128)),
                                    h_slice(qT, h, slice(q_lo, q_lo + wj)),
                                    start=True, stop=True)
                                pss.append(ps)
                            for par, h in enumerate(heads):
                                et = esb.tile([128, 3 * W], BF16,
                                              tag=f"e{par}_{jdx % 4}",
                                              name=f"e{par}_{jdx % 4}")
                                nc.scalar.activation(out=et[:, :wj],
                                                     in_=pss[par][:, :wj],
                                                     func=AF.Exp,
                                                     bias=am_sb[:, jdx:jdx + 1],
                                                     scale=ISCALE)
                                nc.vector.tensor_mul(out=et[:, :wj],
                                                     in0=et[:, :wj],
                                                     in1=masks[jdx][:, :wj])
                                e_tiles[(par, j)] = (et, cs)

                        for par, h in enumerate(heads):
                            av = av_ps.tile([65, T], F32, tag=f"av{par}",
                                            name=f"av{par}")
                            for c in range(NCH):
                                for i, j in enumerate((c - 1, c, c + 1)):
                                    et, cs = e_tiles[(par, j)]
                                    off = cs.index(c) * 128
                                    nc.tensor.matmul(
                                        av[:, c * 128:(c + 1) * 128],
                                        v_sb[j + 1][:, h * 65:h * 65 + 65],
                                        et[:, off:off + 128],
                                        start=(i == 0), stop=False)
                                g4 = h % 4
                                nc.tensor.matmul(
                                    av[:, c * 128:(c + 1) * 128],
                                    vg4_sb[g4 * 32:g4 * 32 + 32,
                                           h * 65:h * 65 + 65],
                                    eg_sb[h // 4][g4 * 32:g4 * 32 + 32,
                                                  c * 128:(c + 1) * 128],
                                    start=False, stop=True,
                                    tile_position=(g4 * 32, 0))
                            # divide by sum-exp row (x16 folded via 1/16 col)
                            rcp = rcsb.tile([1, T], F32, tag="rcp", name="rcp")
                            with nc.allow_low_precision(reason="den>=1/16"):
                                nc.vector.reciprocal(out=rcp[:].bitcast(F32R),
                                                     in_=av[64:65, :])
                            bc = bcsb.tile([64, T], F32, tag="bc", name="bc")
                            nc.gpsimd.partition_broadcast(bc[:], rcp[:])
                            p0 = (h % 2) * DH
                            nc.vector.tensor_mul(
                                out=ctx8[p0:p0 + DH, h // 2:h // 2 + 1, :],
                                in0=av[0:DH, :].unsqueeze(1),
                                in1=bc[:].unsqueeze(1))

                        # global-query partials over owned keys
                        for par, h in enumerate(heads):
                            pg = pg_ps.tile([65, NG], F32, tag=f"pg{par}",
                                            name=f"pg{par}")
                            for j in range(NCH):
                                jdx = j + 1
                                ps = sc_ps.tile([128, 3 * W], F32,
                                                tag=f"sc{par}", name=f"sc{par}")
                                nc.tensor.matmul(
                                    ps[:, :NG],
                                    h_slice(kT, h, slice(jdx * 128, jdx * 128 + 128)),
                                    h_slice(qgT, h, slice(0, NG)),
                                    start=True, stop=True)
                                ep = epsb.tile([128, NG], BF16, tag=f"ep{par}",
                                               name=f"ep{par}")
                                nc.scalar.activation(out=ep, in_=ps[:, :NG],
                                                     func=AF.Exp,
                                                     bias=am_sb[:, jdx:jdx + 1],
                                                     scale=ISCALE)
                                nc.tensor.matmul(pg[:],
                                                 v_sb[jdx][:, h * 65:h * 65 + 65],
                                                 ep[:], start=(j == 0),
                                                 stop=(j == NCH - 1))
                            ogt = ogsb.tile([65, NG], F32, tag=f"og{par}",
                                            name=f"og{par}")
                            nc.vector.tensor_copy(out=ogt, in_=pg)
                            nc.sync.dma_start(out=ap["og"][h], in_=ogt)

        if stop_after == "attn":
            dummy_out(pool_ctx, og_too=False)
            return
        # ---------------- Wo projection (fp8 DR) + residual + LN1 ----------
        with tc.tile_pool(name="wo_load", bufs=1) as wo_load, \
             tc.tile_pool(name="u_sb", bufs=1) as u_sb, \
             tc.tile_pool(name="ln_sb", bufs=1) as ln_sb, \
             tc.tile_pool(name="wo_ps", bufs=2, space="PSUM") as wo_ps, \
             tc.tile_pool(name="row_ps", bufs=1, space="PSUM") as row_ps:
            wo = wo_load.tile([128, KD, D], FP8, tag="wo", name="wo")
            nc.sync.dma_start(out=wo,
                              in_=ap["Wo8"].rearrange("(k p) o -> p k o", p=128))
            for c0 in range(0, T, HALF):
                u_tiles = []
                for o in range(KD):
                    ps = wo_ps.tile([128, HALF], F32, tag="wops", name="wops")
                    for jp in range(KD // 2):
                        nc.tensor.matmul(
                            ps[:], wo[:, 2 * jp:2 * jp + 2, o * 128:(o + 1) * 128],
                            ctx8[:, 2 * jp:2 * jp + 2, c0:c0 + HALF],
                            start=(jp == 0), stop=(jp == KD // 2 - 1),
                            perf_mode=DR)
                    u = u_sb.tile([128, HALF], F32, tag=f"u{o}", name=f"u{o}")
                    nc.scalar.activation(out=u[:].bitcast(F32R), in_=ps,
                                         func=AF.Identity,
                                         bias=bo_sb[:, o:o + 1],
                                         scale=1.0 / (W8S * CTXS))
                    nc.vector.tensor_add(out=u[:].bitcast(F32R), in0=u,
                                         in1=xT[o][:, W + c0:W + c0 + HALF])
                    u_tiles.append(u)
                layernorm(u_tiles, g1_sb, None,
                          [attn_out[k][:, c0:c0 + HALF] for k in range(KD)],
                          ln_sb, row_ps)
                for k in range(KD):
                    # b1 rides on this copy (attn_out itself stays bias-free;
                    # host folds ln1_b into bo2)
                    nc.scalar.activation(out=attn_outB[:, k, c0:c0 + HALF],
                                         in_=attn_out[k][:, c0:c0 + HALF],
                                         func=AF.Identity,
                                         bias=b1_sb[:, k:k + 1])

    if stop_after == "wo":
        dummy_out(pool_ao, og_too=False)
        return
    # ---------------- FFN ----------------
    with tc.tile_pool(name="u2_sb", bufs=1) as u2_sb:
        u2_all = {}
        with tc.tile_pool(name="wi_load", bufs=1) as wi_load, \
             tc.tile_pool(name="wo2_load", bufs=1) as wo2_load, \
             tc.tile_pool(name="inter_sb", bufs=3) as inter_sb, \
             tc.tile_pool(name="ffn_ps", bufs=2, space="PSUM") as ffn_ps, \
             tc.tile_pool(name="o2_ps", bufs=1, space="PSUM") as o2_ps:
            if WI_FP8:
                wi = wi_load.tile([128, KD, FF], FP8, tag="wi", name="wi")
                nc.sync.dma_start(
                    out=wi, in_=ap["Wi8"].rearrange("(k p) o -> p k o", p=128))
            else:
                wi = [wi_load.tile([128, FF], BF16, tag=f"wi{k}", name=f"wi{k}")
                      for k in range(KD)]
                for k in range(KD):
                    nc.sync.dma_start(out=wi[k],
                                      in_=ap["Wi"][k * 128:(k + 1) * 128, :])
            if WO2_FP8:
                wo2 = wo2_load.tile([128, KF, D], FP8, tag="wo2", name="wo2")
                nc.sync.dma_start(
                    out=wo2, in_=ap["Wo28"].rearrange("(k p) o -> p k o", p=128))
            else:
                wo2 = [wo2_load.tile([128, D], BF16, tag=f"wo2_{f}",
                                     name=f"wo2_{f}") for f in range(KF)]
                for f in range(KF):
                    nc.sync.dma_start(out=wo2[f],
                                      in_=ap["Wo2"][f * 128:(f + 1) * 128, :])
            for c0 in range(0, T, HALF):
                o2 = o2_ps.tile([128, KD, HALF], F32, tag="o2", name="o2")
                for fp2 in range(KF // 2):
                    it2 = inter_sb.tile([128, 2, HALF], FP8 if WO2_FP8 else BF16,
                                        tag="it2", name="it2")
                    for par in range(2):
                        f = 2 * fp2 + par
                        ps = ffn_ps.tile([128, HALF], F32, tag="ffn", name="ffn")
                        if WI_FP8:
                            for jp in range(KD // 2):
                                nc.tensor.matmul(
                                    ps[:],
                                    wi[:, 2 * jp:2 * jp + 2, f * 128:(f + 1) * 128],
                                    attn_outB[:, 2 * jp:2 * jp + 2, c0:c0 + HALF],
                                    start=(jp == 0), stop=(jp == KD // 2 - 1),
                                    perf_mode=DR)
                        else:
                            for k in range(KD):
                                nc.tensor.matmul(
                                    ps[:], wi[k][:, f * 128:(f + 1) * 128],
                                    attn_outB[:, k, c0:c0 + HALF],
                                    start=(k == 0), stop=(k == KD - 1))
                        nc.scalar.activation(out=it2[:, par, :], in_=ps,
                                             func=gelu_f,
                                             bias=bi_sb[:, f:f + 1])
                    if WO2_FP8:
                        for o in range(KD):
                            nc.tensor.matmul(
                                o2[:, o, :],
                                wo2[:, 2 * fp2:2 * fp2 + 2, o * 128:(o + 1) * 128],
                                it2[:, :, :],
                                start=(fp2 == 0), stop=(fp2 == KF // 2 - 1),
                                perf_mode=DR)
                    else:
                        for par in range(2):
                            f = 2 * fp2 + par
                            for o in range(KD):
                                nc.tensor.matmul(
                                    o2[:, o, :],
                                    wo2[f][:, o * 128:(o + 1) * 128],
                                    it2[:, par, :],
                                    start=(f == 0), stop=(f == KF - 1))
                for o in range(KD):
                    u = u2_sb.tile([128, HALF], F32, tag=f"u2_{c0}_{o}",
                                   name=f"u2_{c0}_{o}")
                    nc.scalar.activation(out=u[:].bitcast(F32R), in_=o2[:, o, :],
                                         func=AF.Identity,
                                         bias=bo2_sb[:, o:o + 1],
                                         scale=(1.0 / W8S) if WO2_FP8 else 1.0)
                    nc.vector.tensor_add(out=u[:].bitcast(F32R), in0=u,
                                         in1=attn_out[o][:, c0:c0 + HALF])
                    u2_all[(c0, o)] = u

        # ---------------- LN2 -> output DMA ----------------
        with tc.tile_pool(name="ln_sb2", bufs=1) as ln_sb2, \
             tc.tile_pool(name="out_sb", bufs=2) as out_sb, \
             tc.tile_pool(name="row_ps2", bufs=1, space="PSUM") as row_ps2:
            for c0 in range(0, T, HALF):
                dest = [out_sb.tile([128, HALF], F32, tag=f"ot{k}", name=f"ot{k}")
                        for k in range(KD)]
                layernorm([u2_all[(c0, o)] for o in range(KD)], g2_sb, b2_sb,
                          [d[:] for d in dest], ln_sb2, row_ps2)
                for k in range(KD):
                    nc.sync.dma_start(out=ap["outT"][k * 128:(k + 1) * 128,
                                                     c0:c0 + HALF], in_=dest[k])


def build_nc(sim_mode=False, repeat=1, stop_after=None):
    from contextlib import ExitStack
    nc = bacc.Bacc("TRN2", target_bir_lowering=False, debug=False)
    ap = {}
    ap["x_halo"] = nc.dram_tensor("x_halo", [TH, D], F32, kind="ExternalInput").ap()
    ap["x_glob"] = nc.dram_tensor("x_glob", [NG, D], F32, kind="ExternalInput").ap()
    ap["am_halo"] = nc.dram_tensor("am_halo", [TH], F32, kind="ExternalInput").ap()
    ap["am_glob4"] = nc.dram_tensor("am_glob4", [128], F32,
                                    kind="ExternalInput").ap()
    ap["mask_all"] = nc.dram_tensor("mask_all", [NJ, 128, 3 * W], BF16,
                                    kind="ExternalInput").ap()
    for n, sh in (("Wq8", [D, D]), ("Wk8", [D, D]), ("Wv8", [D, D]),
                  ("Wo8", [D, D])):
        ap[n] = nc.dram_tensor(n, sh, FP8, kind="ExternalInput").ap()
    if WI_FP8:
        ap["Wi8"] = nc.dram_tensor("Wi8", [D, FF], FP8, kind="ExternalInput").ap()
    else:
        ap["Wi"] = nc.dram_tensor("Wi", [D, FF], BF16, kind="ExternalInput").ap()
    if WO2_FP8:
        ap["Wo28"] = nc.dram_tensor("Wo28", [FF, D], FP8,
                                    kind="ExternalInput").ap()
    else:
        ap["Wo2"] = nc.dram_tensor("Wo2", [FF, D], BF16,
                                   kind="ExternalInput").ap()
    for n, sh in (("bq", [D]), ("bk", [D]), ("bv", [D]), ("bo", [D]),
                  ("bi", [FF]), ("bo2", [D]), ("ln1_g", [D]), ("ln1_b", [D]),
                  ("ln2_g", [D]), ("ln2_b", [D])):
        ap[n] = nc.dram_tensor(n, sh, F32, kind="ExternalInput").ap()
    ap["outT"] = nc.dram_tensor("outT", [D, T], F32, kind="ExternalOutput").ap()
    ap["og"] = nc.dram_tensor("og", [H, 65, NG], F32, kind="ExternalOutput").ap()

    with tile.TileContext(nc) as tc:
        if repeat > 1:
            def body(i):
                with ExitStack() as c2:
                    build_body(nc, tc, ap, c2, sim_mode, stop_after)
            tc.For_i_unrolled(0, repeat, 1, body, max_unroll=1)
        else:
            with ExitStack() as c2:
                build_body(nc, tc, ap, c2, sim_mode, stop_after)
    nc.compile()
    return nc


# ---------------- host side ----------------

def _fp8(x, scale):
    return np.asarray(np.asarray(x, np.float32) * scale,
                      ml_dtypes.float8_e4m3)


def shard_inputs(inputs):
    hs = np.asarray(inputs["hidden_states"], np.float32)
    am = np.asarray(inputs["attention_mask"], np.float32)
    shared = {}
    for n in ("bq", "bk", "bv", "bo", "bi", "ln1_g", "ln1_b", "ln2_g", "ln2_b"):
        shared[n] = np.ascontiguousarray(np.asarray(inputs[n], np.float32))
    # ln1_b folds into the u2 residual bias (attn_out tile is bias-free)
    shared["bo2"] = np.ascontiguousarray(
        np.asarray(inputs["bo2"], np.float32)
        + np.asarray(inputs["ln1_b"], np.float32))
    for n in ("Wq", "Wk", "Wv", "Wo"):
        shared[n + "8"] = _fp8(inputs[n], W8S)
    if WI_FP8:
        shared["Wi8"] = _fp8(inputs["Wi"], W8S)
    else:
        shared["Wi"] = np.asarray(inputs["Wi"], np.float32).astype(
            ml_dtypes.bfloat16)
    if WO2_FP8:
        shared["Wo28"] = _fp8(inputs["Wo2"], W8S)
    else:
        shared["Wo2"] = np.asarray(inputs["Wo2"], np.float32).astype(
            ml_dtypes.bfloat16)
    in_maps = []
    for core in range(N_CORES):
        b, q0 = core // 4, (core % 4) * T
        xh = np.zeros((TH, D), np.float32)
        amh = np.zeros((TH,), np.float32)
        lo, hi = q0 - W, q0 + T + W
        slo, shi = max(lo, 0), min(hi, S)
        xh[slo - lo:shi - lo] = hs[b, slo:shi]
        amh[slo - lo:shi - lo] = am[b, slo:shi]
        mask = np.zeros((NJ, 128, 3 * W), np.float32)
        for j in range(-1, NCH + 1):
            cs = [c for c in (j - 1, j, j + 1) if 0 <= c < NCH]
            kpos = q0 + j * 128 + np.arange(128)[:, None]
            for i, c in enumerate(cs):
                qpos = q0 + c * 128 + np.arange(128)[None, :]
                valid = (np.abs(kpos - qpos) <= W) & (kpos >= NG) & (kpos >= 0) \
                    & (kpos < S)
                mask[j + 1, :, i * 128:(i + 1) * 128] = valid
        m = {"x_halo": xh, "x_glob": np.ascontiguousarray(hs[b, :NG]),
             "am_halo": amh,
             "am_glob4": np.ascontiguousarray(np.tile(am[b, :NG], 4)),
             "mask_all": mask.astype(ml_dtypes.bfloat16)}
        m.update(shared)
        in_maps.append(m)
    return in_maps


def _np_layernorm(x, g, b):
    mu = x.mean(-1, keepdims=True)
    var = ((x - mu) ** 2).mean(-1, keepdims=True)
    return (x - mu) / np.sqrt(var + EPS) * g + b


def _np_gelu(x):
    from scipy.special import erf
    return x * 0.5 * (1.0 + erf(x / np.sqrt(2.0)))


def host_tail(inputs, og_by_core, sim_mode=False):
    """Combine global-query flash partials; dense tail for the global rows.
    og denominators carry the 1/16 ones-column scale."""
    hs = np.asarray(inputs["hidden_states"], np.float64)
    rows = np.zeros((B, NG, D))
    for b in range(B):
        o = sum(np.asarray(og_by_core[4 * b + c], np.float64) for c in range(4))
        gctx = o[:, :DH, :] / (o[:, 64:65, :] * CTXS)       # [H, DH, NG]
        gctx = gctx.transpose(2, 0, 1).reshape(NG, D)
        u = gctx @ np.asarray(inputs["Wo"], np.float64) \
            + np.asarray(inputs["bo"], np.float64) + hs[b, :NG]
        a = _np_layernorm(u, np.asarray(inputs["ln1_g"], np.float64),
                          np.asarray(inputs["ln1_b"], np.float64))
        inter = a @ np.asarray(inputs["Wi"], np.float64) \
            + np.asarray(inputs["bi"], np.float64)
        if not sim_mode:
            inter = _np_gelu(inter)
        u2 = inter @ np.asarray(inputs["Wo2"], np.float64) \
            + np.asarray(inputs["bo2"], np.float64) + a
        rows[b] = _np_layernorm(u2, np.asarray(inputs["ln2_g"], np.float64),
                                np.asarray(inputs["ln2_b"], np.float64))
    return rows.astype(np.float32)


def assemble(inputs, results, sim_mode=False):
    out = np.zeros((B, S, D), np.float32)
    for core in range(N_CORES):
        b, q0 = core // 4, (core % 4) * T
        out[b, q0:q0 + T] = np.asarray(results[core]["outT"]).T
    out[:, :NG] = host_tail(inputs, [results[c]["og"] for c in range(N_CORES)],
                            sim_mode)
    return out


def kernel(**inputs):
    from concourse import bass_utils
    if "nc" not in _nc_cache:
        _nc_cache["nc"] = build_nc()
    nc = _nc_cache["nc"]
    in_maps = shard_inputs(inputs)
    res = bass_utils.run_bass_kernel_spmd(nc, in_maps, core_ids=list(range(N_CORES)))
    return assemble(inputs, res.results)
